# revision 42
# baseline (speedup 1.0000x reference)
"""Trainium2 Bass kernel for nn_CNNNer (sparse band biaffine NER scorer).

Math collapse (everything after the GELU stage is linear):
  head = gelu(state@Wh+bh) ++ [1]          (features i = 0..200, i=200 is the 1)
  tail = gelu(state@Wt+bt) ++ [1]
  band[n,r,k] = head[n]^T U''_k tail[m],  m = n+r-64
      with U''_k = U_k + e_200 Wtp[k,:] + Whp[k,:]^T e_200^T
  scores'[n,r,t] = head[n]^T UW_t tail[m],  UW_t = sum_k Wd[k,t] U''_k
      (precomputed on host, [9,201,201]); scores = scores' + bd.
  Pad masking only ever zeroes whole band entries -> masked scores equal bd
  exactly, so masking moves to the host entirely (device computes garbage in
  masked slots; finite, overwritten on host).

Device work per core (8 cores; core = (batch b, query quarter), 256 queries,
window of NW=384 key positions), all matmuls bf16 with fp32 PSUM accumulate:
  MLP:   headT[f, x] (queries, 256 cols), tailT[f, m] (384 cols), f = 201
         (128 + 73 partition tiles; row 200 memset to 1.0).
  A:     uhT_t[j, x] = sum_i UW[t,i,j] headT[i,x]   (9 tags, 36 matmuls)
  B:     S_t[m, x]   = sum_j tailT[j, m] uhT_t[j, x], computed as 4 groups
         (m-block 0 x-lo, m-block 1 x-lo, m-block 1 x-hi, m-block 2 x-hi),
         tail slice stationary, uh (t,x)-chunks of 512 moving.
Output sout[4, 128, 1152] bf16 = [group][m][(t,x)]; host extracts the
129-wide band diagonals, applies pad mask and + bd.
"""

import os

import numpy as np

B, N, HID = 2, 1024, 768
BSZ = 200
W = 64
TAGS = 9
F = BSZ + 1  # 201 features incl the ones column
NQ = 256  # queries per core
NW = NQ + 2 * W  # 384 window positions per core
R = 2 * W + 1  # 129 band offsets
NCORES = 8
I2 = F - 128  # 73: second feature tile rows (incl ones row at local 72)
F2 = BSZ - 128  # 72: second MLP output tile rows
GSZ = TAGS * 128  # 1152: per-group output elems per partition

_cache: dict = {}


def io_dt_name():
    return os.environ.get("BASSK_IO_DT", "bf16")


def _build_nc():
    import concourse.bass as bass
    import concourse.mybir as mybir
    import concourse.tile as tile
    from concourse import bacc

    dt = mybir.dt
    f32 = dt.float32
    io = {"f32": f32, "f32r": dt.float32r, "bf16": dt.bfloat16}[io_dt_name()]
    nwarm = int(os.environ.get("BASSK_WARM", "10"))

    nc = bacc.Bacc(
        "TRN2", target_bir_lowering=False, debug=False, enable_asserts=False
    )
    # All dram layouts are partition-major with per-partition contiguous runs
    # of 2.3-4.6KB so DMA descriptors are fat and stripe across the 16 DMA
    # engines.
    # One fat blob per DMA queue (chain latency is ~descriptor-count bound,
    # so fewer/fatter per-partition descriptors win). Blob A = x ++ wh,
    # blob B = wt ++ uw[0:2412], blob C = uw[2412:3618]. uw is UW packed two
    # i-rows per partition: partition p holds flat row p (elems 0:1809) and
    # row 128+p (1809:3618, zeros for p >= 73); the B/C split at 2412 falls
    # on a tag boundary so no matmul slice straddles it. The 8-elem dram pad
    # per partition blocks cross-partition descriptor merging.
    tAd = nc.dram_tensor("tAd", [128, 3512], io, kind="ExternalInput").ap()
    tBd = nc.dram_tensor("tBd", [128, 3620], io, kind="ExternalInput").ap()
    tCd = nc.dram_tensor("tCd", [128, 1216], io, kind="ExternalInput").ap()
    # bias row [bh(200) pad | bt(200) pad], one descriptor; accumulated into
    # the MLP psum as a rank-1 matmul against the ones row before the weight
    # chunks arrive (a [128,4] dram layout would need a 128-descriptor
    # chain, ~3.7us of queue latency for 2KB)
    biasr = nc.dram_tensor("biasr", [1, 512], io, kind="ExternalInput").ap()
    onesd = nc.dram_tensor("onesd", [1, NW], io, kind="ExternalInput").ap()
    sout = nc.dram_tensor("sout", [4, 128, GSZ], io, kind="ExternalOutput").ap()

    gelu = {
        "gelu": mybir.ActivationFunctionType.Gelu,
        "identity": mybir.ActivationFunctionType.Identity,
    }[os.environ.get("BASSK_ACT", "gelu")]
    copyf = mybir.ActivationFunctionType.Copy

    with tile.TileContext(nc) as tc:
        with (
            tc.tile_pool(name="sb", bufs=1) as sb,
            tc.tile_pool(name="ps", bufs=1, space="PSUM") as ps,
        ):
            tA = sb.tile([128, 3504], io)  # x flat (2304) ++ wh flat (1200)
            tB = sb.tile([128, 3612], io)  # wt flat (1200) ++ uw[0:2412]
            tC = sb.tile([128, 1206], io)  # uw[2412:3618]
            br_sb = sb.tile([1, 512], io)
            ones_sb = sb.tile([1, NW], io)

            def xs(ht, c0, n):
                return tA[:, ht * NW + c0 : ht * NW + c0 + n]

            def whs(ht, f0, fw):
                o = 2304 + ht * BSZ + f0
                return tA[:, o : o + fw]

            def wts(ht, f0, fw):
                o = ht * BSZ + f0
                return tB[:, o : o + fw]

            def uws(off, ip, jw):
                if off < 2412:
                    return tB[0:ip, 1200 + off : 1200 + off + jw]
                return tC[0:ip, off - 2412 : off - 2412 + jw]
            headT1 = sb.tile([128, NQ], io)
            headT2 = sb.tile([I2, NQ], io)
            tailT1 = sb.tile([128, NW], io)
            tailT2 = sb.tile([I2, NW], io)
            uh1 = sb.tile([128, TAGS, NQ], io)
            uh2 = sb.tile([I2, TAGS, NQ], io)
            s_sb = sb.tile([128, 4, GSZ], io)
            warm = sb.tile([128, 512], io)

            # ---- loads: head inputs on sync, tail weights on scalar, uw
            # chains on gpsimd — three queues streaming concurrently, each
            # chain 128 fat descriptors, ordered by first use ----
            nc.sync.dma_start(out=headT2[F2:I2, :], in_=onesd[:, 0:NQ])
            nc.sync.dma_start(out=tailT2[F2:I2, :], in_=onesd)
            nc.sync.dma_start(out=ones_sb, in_=onesd)
            nc.sync.dma_start(out=br_sb, in_=biasr)
            nc.sync.dma_start(out=tA, in_=tAd[:, 0:3504])
            nc.scalar.dma_start(out=tB, in_=tBd[:, 0:3612])
            nc.gpsimd.dma_start(out=tC, in_=tCd[:, 0:1206])

            nc.vector.memset(warm, 0.5)

            # PSUM: tag "u" = [128, 1024]-f32 two-bank slots (3 bufs), tag
            # "a" = one-bank slots (2 bufs) — 8 banks total. Accumulation
            # groups always start at a bank boundary (a group's start-flag
            # zeroes its whole 2KB bank).
            def pslot():
                return ps.tile([128, 2, 512], f32, tag="u", bufs=3, name="pu")

            def pslot1():
                return ps.tile([128, 512], f32, tag="a", bufs=2, name="pa")

            # ---- PE clock warmup: dependency-free matmuls run while the
            # input DMAs stream, ramping the tensor engine out of its low
            # p-state before the real work arrives ----
            if nwarm:
                pw = pslot()
                for k in range(nwarm):
                    nc.tensor.matmul(
                        pw[:, 0, :], warm[:, 0:128], warm,
                        start=(k == 0), stop=(k == nwarm - 1),
                    )

            # ---- MLPs: o = gelu(W^T x + b); bias enters as a rank-1
            # matmul (ones x bias row) accumulated before the W chunks, so
            # it costs tensor time only inside the DMA wait. f1 before f2
            # so the f1 gelu runs while f2 still accumulates; head before
            # tail so step A starts as soon as possible. ----
            pm_h = pslot()
            pm_t = pslot()
            nc.tensor.matmul(
                pm_h[:, 0, 0:NQ], br_sb[:, 0:128], ones_sb[:, 0:NQ],
                start=True, stop=False,
            )
            nc.tensor.matmul(
                pm_h[0:F2, 1, 0:NQ], br_sb[:, 128:BSZ], ones_sb[:, 0:NQ],
                start=True, stop=False,
            )
            nc.tensor.matmul(
                pm_t[:, 0, 0:NW], br_sb[:, 256:384], ones_sb,
                start=True, stop=False,
            )
            nc.tensor.matmul(
                pm_t[0:F2, 1, 0:NW], br_sb[:, 384:456], ones_sb,
                start=True, stop=False,
            )
            for f0, fw, sl in ((0, 128, 0), (128, F2, 1)):
                for ht in range(6):
                    nc.tensor.matmul(
                        pm_h[0:fw, sl, 0:NQ], whs(ht, f0, fw),
                        xs(ht, W, NQ), start=False, stop=(ht == 5),
                    )
            nc.scalar.activation(
                out=headT1, in_=pm_h[:, 0, 0:NQ], func=gelu
            )
            nc.scalar.activation(
                out=headT2[0:F2, :], in_=pm_h[0:F2, 1, 0:NQ], func=gelu
            )
            for f0, fw, sl in ((0, 128, 0), (128, F2, 1)):
                for ht in range(6):
                    nc.tensor.matmul(
                        pm_t[0:fw, sl, 0:NW], wts(ht, f0, fw), xs(ht, 0, NW),
                        start=False, stop=(ht == 5),
                    )
            nc.scalar.activation(
                out=tailT1, in_=pm_t[:, 0, 0:NW], func=gelu
            )
            nc.scalar.activation(
                out=tailT2[0:F2, :], in_=pm_t[0:F2, 1, 0:NW], func=gelu
            )

            # ---- step A: uhT_t[j, x] = sum_i UW[t,i,j] headT[i,x]; tag
            # pairs share one PSUM bank so copies are 512 wide ----
            # only vector (DVE) and scalar (ACT) can read PSUM
            def pcopy(k, out, in_):
                if k % 2:
                    nc.scalar.activation(out=out, in_=in_, func=copyf)
                else:
                    nc.vector.tensor_copy(out, in_)

            TF = TAGS * F
            for jt, (jw, j0, uh) in enumerate(((128, 0, uh1), (I2, 128, uh2))):
                for tp in range(5):
                    t0, tn = 2 * tp, min(2, TAGS - 2 * tp)
                    pa = pslot()
                    for tt in range(tn):
                        off = (t0 + tt) * F + j0
                        for it, (ip, h_sb) in enumerate(
                            ((128, headT1), (I2, headT2))
                        ):
                            nc.tensor.matmul(
                                pa[0:jw, tt, 0:NQ],
                                uws(it * TF + off, ip, jw),
                                h_sb,
                                start=(it == 0), stop=(it == 1),
                            )
                    pcopy(
                        jt * 5 + tp,
                        uh[0:jw, t0 : t0 + tn, :], pa[0:jw, 0:tn, 0:NQ]
                    )

            # ---- step B: S[m, (t,x)] = sum_j tailT[j, m] uhT[j, (t,x)];
            # tail slice stationary, uh chunks of 512 moving, 4 groups ----
            # groups: (m-block, x-half): (0, lo), (1, lo), (1, hi), (2, hi)
            groups = ((0, 0), (1, 0), (1, 1), (2, 1))
            # chunks of the 9*128 free dim: tags 0-3 and 4-7 in the two
            # banks of a "u" slot, tag 8 in an "a" slot
            sq = (nc.sync, nc.scalar, nc.gpsimd, nc.sync)
            for g, (mb, xh) in enumerate(groups):
                m0 = mb * 128
                x0 = xh * 128
                pb = pslot()
                pc = pslot1()
                for jt, (jw, tl, uh) in enumerate(
                    ((128, tailT1, uh1), (I2, tailT2, uh2))
                ):
                    for po, ct, cn in (
                        (pb[:, 0, 0:512], 0, 4),
                        (pb[:, 1, 0:512], 4, 4),
                        (pc[:, 0:128], 8, 1),
                    ):
                        nc.tensor.matmul(
                            po,
                            tl[0:jw, m0 : m0 + 128],
                            uh[0:jw, ct : ct + cn, x0 : x0 + 128],
                            start=(jt == 0), stop=(jt == 1),
                        )
                pcopy(g, s_sb[:, g, 0:1024], pb.rearrange("p a b -> p (a b)"))
                pcopy(g + 1, s_sb[:, g, 1024:GSZ], pc[:, 0:128])
                sq[g].dma_start(out=sout[g], in_=s_sb[:, g, :])

    nc.compile()
    return nc


def _np_io_dt():
    if io_dt_name() == "bf16":
        import ml_dtypes

        return ml_dtypes.bfloat16
    return np.float32


def _get_nc():
    key = "nc-" + io_dt_name() + os.environ.get("BASSK_WARM", "10")
    if key not in _cache:
        _cache[key] = _build_nc()
    return _cache[key]


def _install_ntff_hook():
    """Profiling-only (BASSK_TRACE=1): provide antenv.axon_hooks if the
    image lacks it, wired to the libaxon NTFF capture via ctypes."""
    import sys
    import types

    try:
        from antenv.axon_hooks import get_axon_ntff_profile_hook  # noqa: F401

        return
    except ImportError:
        pass
    from trn_agent_boot.trn_boot import _ntff_profile_via_ctypes

    hook = _ntff_profile_via_ctypes("/opt/axon/libaxon_pjrt.so")
    mod = types.ModuleType("antenv.axon_hooks")
    mod._hook = hook
    mod.get_axon_ntff_profile_hook = lambda: mod._hook
    mod.set_axon_ntff_profile_hook = lambda h: setattr(mod, "_hook", h)
    sys.modules["antenv.axon_hooks"] = mod


def _host_prep(state, Wh, bh, Wt, bt, U, Wcat, Wd):
    """Fold U/Wcat/Wd into UW[9,201,201] and build per-core inputs."""
    iodt = _np_io_dt()
    Whp = Wcat[:, :F]  # [K, 201]
    Wtp = Wcat[:, F:]  # [K, 201]
    U2 = U.astype(np.float64).copy()
    U2[:, F - 1, :] += Wtp  # head ones-row picks up the tail term
    U2[:, :, F - 1] += Whp  # tail ones-col picks up the head term
    UW = np.einsum("kt,kij->tij", Wd.astype(np.float64), U2).astype(np.float32)
    UWt = np.ascontiguousarray(UW.transpose(1, 0, 2))  # [i, t, j]

    whflat = (
        Wh.reshape(6, 128, BSZ).transpose(1, 0, 2).reshape(128, 1200)
    )
    wtflat = (
        Wt.reshape(6, 128, BSZ).transpose(1, 0, 2).reshape(128, 1200)
    )
    biasr = np.zeros((1, 512), np.float32)
    biasr[0, 0:BSZ] = bh
    biasr[0, 256 : 256 + BSZ] = bt
    uwflat = UWt.reshape(F, TAGS * F)  # [i, 1809]
    uwcat = np.zeros((128, 2 * TAGS * F), np.float32)
    uwcat[:, 0 : TAGS * F] = uwflat[0:128]
    uwcat[0:I2, TAGS * F :] = uwflat[128:F]
    tBd = np.zeros((128, 3620), iodt)
    tBd[:, 0:1200] = wtflat.astype(iodt)
    tBd[:, 1200:3612] = uwcat[:, 0:2412].astype(iodt)
    tCd = np.zeros((128, 1216), iodt)
    tCd[:, 0:1206] = uwcat[:, 2412:3618].astype(iodt)
    onesd = np.ones((1, NW), iodt)

    in_maps = []
    for b in range(B):
        for qi in range(N // NQ):
            lo = qi * NQ - W
            xw = np.zeros((NW, HID), np.float32)
            s, e = max(lo, 0), min(lo + NW, N)
            xw[s - lo : e - lo] = state[b, s:e]
            tAd = np.zeros((128, 3512), iodt)
            tAd[:, 0:2304] = (
                xw.T.reshape(6, 128, NW).transpose(1, 0, 2).reshape(128, 2304)
            ).astype(iodt)
            tAd[:, 2304:3504] = whflat.astype(iodt)
            in_maps.append(
                {
                    "tAd": tAd,
                    "tBd": tBd,
                    "tCd": tCd,
                    "biasr": biasr.astype(iodt),
                    "onesd": onesd,
                }
            )
    return in_maps


def _assemble(outs, bd, lengths):
    """outs: NCORES arrays [4, 128, TAGS*128] -> scores [B, N, R, TAGS]."""
    n_ar = np.arange(N)
    offs = np.arange(R) - W
    j_idx = n_ar[:, None] + offs[None, :]  # [N, R]
    in_range = (j_idx >= 0) & (j_idx < N)
    key_ok = in_range[None] & (j_idx[None] < lengths[:, None, None])
    q_ok = n_ar[None, :] < lengths[:, None]
    pad = ~(key_ok & q_ok[:, :, None])  # [B, N, R]

    xx = np.arange(128)
    idx = (xx[:, None] + np.arange(R)[None, :])[:, :, None]  # [128, R, 1]
    scores = np.empty((B, N, R, TAGS), np.float32)
    for c, S in enumerate(outs):
        b, qi = divmod(c, N // NQ)
        q0 = qi * NQ
        G = S.astype(np.float32).reshape(4, 128, TAGS, 128)
        for half in range(2):
            H = np.concatenate(
                (G[2 * half], G[2 * half + 1]), axis=0
            )  # [256 m, TAGS, 128 xx]
            T = H.transpose(2, 0, 1)  # [xx, m, t]
            band = np.take_along_axis(T, idx, axis=1)  # [128, R, TAGS]
            scores[b, q0 + 128 * half : q0 + 128 * (half + 1)] = band
    scores = np.where(pad[..., None], 0.0, scores) + bd.astype(np.float32)
    return np.where(np.isfinite(scores), scores, 0.0).astype(np.float32)


def kernel(**inputs):
    state = np.asarray(inputs["state"], np.float32)
    lengths = np.asarray(inputs["lengths"]).astype(np.int64)
    Wh = np.ascontiguousarray(np.asarray(inputs["Wh"], np.float32))
    bh = np.asarray(inputs["bh"], np.float32)
    Wt = np.ascontiguousarray(np.asarray(inputs["Wt"], np.float32))
    bt = np.asarray(inputs["bt"], np.float32)
    U = np.asarray(inputs["U"], np.float32)
    Wcat = np.asarray(inputs["Wcat"], np.float32)
    Wd = np.asarray(inputs["Wd"], np.float32)
    bd = np.asarray(inputs["bd"], np.float32)

    in_maps = _host_prep(state, Wh, bh, Wt, bt, U, Wcat, Wd)
    nc = _get_nc()

    if os.environ.get("BASSK_SIM"):
        from concourse.bass_interp import CoreSim

        outs = []
        for im in in_maps:
            sim = CoreSim(nc, trace=False)
            for k, v in im.items():
                sim.tensor(k)[:] = v
            sim.simulate()
            outs.append(sim.tensor("sout").copy())
    else:
        trace = bool(os.environ.get("BASSK_TRACE"))
        if trace:
            _install_ntff_hook()
        from concourse.bass_utils import run_bass_kernel_spmd

        try:
            res = run_bass_kernel_spmd(
                nc, in_maps, core_ids=list(range(NCORES)), trace=trace
            )
        except Exception:
            # transient NRT/device hiccups recover on a fresh attempt
            import time

            time.sleep(2.0)
            res = run_bass_kernel_spmd(
                nc, in_maps, core_ids=list(range(NCORES)), trace=trace
            )
        _cache["last_result"] = res
        outs = [r["sout"] for r in res.results]

    return _assemble(outs, bd, lengths)
